# revision 15
# baseline (speedup 1.0000x reference)
"""Trainium2 Bass kernel for a 2-bit-quantized (DoReFa) ResNet BasicBlock.

Full (unsharded) numpy inputs -> full numpy output.

Design:
  - batch (64) is sharded 8 images/core across 8 NeuronCores (pure data
    parallel, weights/BN params replicated).
  - 2-bit quantization makes every conv input an exact small integer:
    acts*3 in {0..3}, weights*3 in {-3,-1,1,3}. Those are exact in fp8e4
    and the PE accumulates in fp32, so both convs are bit-exact integer
    matmuls at fp8 DoubleRow speed. All scale factors (the /9, BN
    scale/shift) fold into per-channel epilogue constants on host.
  - each 3x3 conv = 9 shifted matmuls accumulated in PSUM over padded
    30-wide activation planes. fp8 DoubleRow contracts all 256 input
    channels per matmul. The moving operand is a flat 418-element run
    across 14 plane rows; 2 garbage columns per row land in ignored psum
    columns.
  - activations everywhere use the is_gt coding qa = sum_k 1[v > t_k] in
    {0..3} with zero padding (zero == quantized zero), so conv1 and conv2
    use the same per-channel affine folds and no border corrections.
  - epilogues are fused via tensor_scalar / scalar_tensor_tensor with
    per-partition AP scalars:
      conv1: qa2 = ((ps>t1a) + (ps>t1b)) + (ps>t1c)   -> 3 DVE ops
      conv2: v = ps*s2 + x (STT);  out = relu(v + bb2) (ACT bias)
      x-quant: same 3-op is_gt chain (gpsimd for most images).
  - a scratch-fed run of dummy DoubleRow matmuls at kernel start warms the
    PE HAM clock gate so the real matmul stream runs at full rate and
    overlaps the x load + quantization prologue.
  - weight quantization (tanh / global max / round) + BN folding is done
    on host: it is O(weights) = 0.6 MB, vs 118 GFLOP of conv on device.
"""

import os
import sys
import numpy as np


def _install_ntff_hook_shim():
    """Provide antenv.axon_hooks if the image lacks it, so
    run_bass_kernel_spmd(trace=True) can capture NTFF profiles through
    libaxon_pjrt.so. No-op if the real module exists or the .so is absent."""
    try:
        import antenv.axon_hooks  # noqa: F401
        return
    except ImportError:
        pass
    import contextlib
    import ctypes
    import types

    so_path = "/opt/axon/libaxon_pjrt.so"
    _hook = None
    if os.path.exists(so_path):
        try:
            lib = ctypes.CDLL(so_path)
        except OSError:
            lib = None
        if lib is not None and hasattr(lib, "axon_start_nrt_profile"):
            lib.axon_start_nrt_profile.argtypes = [
                ctypes.POINTER(ctypes.c_int64), ctypes.c_size_t]
            lib.axon_start_nrt_profile.restype = ctypes.c_int64
            lib.axon_stop_nrt_profile.argtypes = [ctypes.c_char_p]
            lib.axon_stop_nrt_profile.restype = ctypes.c_int64

            @contextlib.contextmanager
            def _hook(output_dir, device_ids):
                import jax
                jax.devices()
                if device_ids:
                    ids = (ctypes.c_int64 * len(device_ids))(*device_ids)
                    rc = lib.axon_start_nrt_profile(ids, len(device_ids))
                else:
                    rc = lib.axon_start_nrt_profile(None, 0)
                if rc != 0:
                    raise RuntimeError(f"axon_start_nrt_profile rc={rc}")
                try:
                    yield
                finally:
                    n = lib.axon_stop_nrt_profile(str(output_dir).encode())
                    print(f"profile: {n} file(s) written to {output_dir}",
                          file=sys.stderr)

    mod = types.ModuleType("antenv.axon_hooks")
    mod.get_axon_ntff_profile_hook = lambda: _hook
    mod.set_axon_ntff_profile_hook = lambda h: None
    sys.modules["antenv.axon_hooks"] = mod


NCORES = 8
NPER = 8          # images per core
C = 256
NCH = 2           # channel chunks of 128
H = W = 28
PH = H + 2        # padded plane 30x30
PW = 30           # plane row stride
QSTR = 960        # allocated plane stride (16B-aligned, >= PH*PW)
HALF = 14         # rows per psum tile
RUN = (HALF - 1) * PW + W   # 418-element flat moving-run per matmul
PSF = HALF * PW   # 420 psum columns (cols 28..29 of each row are garbage)
BN_EPS = 1e-5
NWARM = int(os.environ.get("KERNEL_NWARM", "12"))   # PE warm-up matmuls
XQV = int(os.environ.get("KERNEL_XQV", "2"))        # imgs with x-quant on DVE


def _quant_weight3(w):
    """Replicate reference _quant_weight in f32, scaled by 3 -> {-3,-1,1,3}."""
    w = np.asarray(w, np.float32)
    t = np.tanh(w)
    m = np.max(np.abs(t))
    t2 = t / (np.float32(2.0) * m) + np.float32(0.5)
    k = np.round(t2 * np.float32(3.0))          # round-half-even == jnp.round
    return (2.0 * k - 3.0).astype(np.float32)


def _fold_bn(g, b, m, v):
    inv = np.asarray(g, np.float64) / np.sqrt(np.asarray(v, np.float64) + BN_EPS)
    beta = np.asarray(b, np.float64) - np.asarray(m, np.float64) * inv
    return inv, beta


def _w_tiles(qw3, dt):
    # [O, I, 3, 3] -> [p=128, ci=2, k=9, O=256] so lhsT slices are
    # [128, 2, 128] interleaved chunks (fp8 DoubleRow).
    return np.ascontiguousarray(
        np.transpose(qw3.reshape(C, NCH, 128, 9), (2, 1, 3, 0))
    ).astype(dt)


def _host_arrays(w1, g1, b1, m1, v1, w2, g2, b2, m2, v2):
    from concourse import mybir
    qw3_1 = _quant_weight3(w1)
    qw3_2 = _quant_weight3(w2)
    inv1, beta1 = _fold_bn(g1, b1, m1, v1)
    inv2, beta2 = _fold_bn(g2, b2, m2, v2)

    act_np = mybir.dt.np(mybir.dt.float8e4)
    w1t = _w_tiles(qw3_1, act_np)
    w2t = _w_tiles(qw3_2, act_np)

    # conv1 psum P1 = 9*conv_true (exact int); y = P1*inv1/9 + beta1.
    # quant level k iff y > tau_k = (2k-1)/6, i.e. P1 > (tau_k-beta1)*9/inv1
    # (inv1 > 0 given g1=1, v1 > 0), so qa2 = sum_k is_gt(P1, t1_k).
    assert np.all(inv1 > 0), "bn1 scale must be positive for threshold fold"
    taus = np.array([1.0, 3.0, 5.0]) / 6.0
    t1 = ((taus[None, :] - beta1[:, None]) * 9.0 / inv1[:, None])  # [C, 3]
    # sign-coded images store S1 = 2*qa1-3 (pad -3), so P1s = 2*P1 - 3*K1f
    # and the thresholds become 2*t1 - 3*K1f per output channel.
    k1f = qw3_1.reshape(C, -1).sum(axis=1)[:, None]                # [C, 1]
    t1s = 2.0 * t1 - 3.0 * k1f

    def fold_t(t):
        return np.ascontiguousarray(
            t.reshape(NCH, 128, 3).transpose(1, 0, 2)).astype(np.float32)

    # conv2 on qa2 in {0..3} with zero padding: P2 = 9*conv2_true, so
    # y2 = P2*(inv2/9) + beta2 and out = relu(y2 + x).
    s2 = np.ascontiguousarray(
        (inv2 / 9.0).reshape(NCH, 128).T).astype(np.float32)
    bb2 = np.ascontiguousarray(
        beta2.reshape(NCH, 128).T).astype(np.float32)
    z0 = np.zeros((128, NCH, QSTR), act_np)
    zm3 = np.full((128, NCH, QSTR), -3.0, act_np)
    xqtn = np.broadcast_to(
        -np.array([1.0, 3.0, 5.0], np.float32) / 6.0, (128, 3)).copy()
    return {"w1t": w1t, "w2t": w2t, "t1": fold_t(t1), "t1s": fold_t(t1s),
            "s2": s2, "bb2": bb2, "z0": z0, "zm3": zm3, "xqtn": xqtn}


def _build_program(nper=NPER):
    from concourse import bacc, tile, mybir
    dt = mybir.dt
    dt_act = dt.float8e4
    ALU = mybir.AluOpType

    nc = bacc.Bacc("TRN2", target_bir_lowering=False, debug=False,
                   num_devices=NCORES)
    NP_ = nper

    x_d = nc.dram_tensor("x", [NP_, C, H, W], dt.float32, kind="ExternalInput")
    w1_d = nc.dram_tensor("w1t", [128, NCH, 9, C], dt_act, kind="ExternalInput")
    w2_d = nc.dram_tensor("w2t", [128, NCH, 9, C], dt_act, kind="ExternalInput")
    t1_d = nc.dram_tensor("t1", [128, NCH, 3], dt.float32, kind="ExternalInput")
    t1s_d = nc.dram_tensor("t1s", [128, NCH, 3], dt.float32,
                           kind="ExternalInput")
    s2_d = nc.dram_tensor("s2", [128, NCH], dt.float32, kind="ExternalInput")
    b2_d = nc.dram_tensor("bb2", [128, NCH], dt.float32, kind="ExternalInput")
    z0_d = nc.dram_tensor("z0", [128, NCH, QSTR], dt_act, kind="ExternalInput")
    zm3_d = nc.dram_tensor("zm3", [128, NCH, QSTR], dt_act,
                           kind="ExternalInput")
    xqtn_d = nc.dram_tensor("xqtn", [128, 3], dt.float32,
                            kind="ExternalInput")
    y_d = nc.dram_tensor("y", [NP_, C, H, W], dt.float32, kind="ExternalOutput")

    XQT = [1.0 / 6.0, 3.0 / 6.0, 5.0 / 6.0]   # act-quant thresholds for x

    with tile.TileContext(nc) as tc:
        with (
            tc.tile_pool(name="wpool", bufs=1) as wpool,
            tc.tile_pool(name="xpool", bufs=2 * NP_) as xpool,
            tc.tile_pool(name="qpool", bufs=NP_) as qpool,
            tc.tile_pool(name="mpool", bufs=4) as mpool,
            tc.tile_pool(name="upool", bufs=4) as upool,
            tc.tile_pool(name="opool", bufs=4) as opool,
            tc.tile_pool(name="pspool", bufs=8, space="PSUM") as pspool,
        ):
            w1_sb = wpool.tile([128, NCH, 9, C], dt_act, name="w1sb")
            w2_sb = wpool.tile([128, NCH, 9, C], dt_act, name="w2sb")
            t1_sb = wpool.tile([128, NCH, 3], dt.float32, name="t1sb")
            t1s_sb = wpool.tile([128, NCH, 3], dt.float32, name="t1ssb")
            xqtn_sb = wpool.tile([128, 3], dt.float32, name="xqtnsb")
            s2_sb = wpool.tile([128, NCH], dt.float32, name="s2sb")
            b2_sb = wpool.tile([128, NCH], dt.float32, name="b2sb")
            wscr = wpool.tile([128, NCH, 512], dt_act, name="wscr")
            # zero-padded quantized-activation planes (flat, per image);
            # both hold is_gt sums in {0..3}, zero is the quantized zero.
            qa1 = [qpool.tile([128, NCH, QSTR], dt_act, name=f"qa1_{n}",
                              tag="qa1") for n in range(NP_)]
            qa2 = [qpool.tile([128, NCH, QSTR], dt_act, name=f"qa2_{n}",
                              tag="qa2") for n in range(NP_)]

            def plane(qa_t, j):
                return qa_t[:, j, :].rearrange("p (r c) -> p r c", c=PW)

            # --- PE warm-up: scratch-fed DoubleRow matmuls keep the HAM
            # clock gate open while the x prologue runs, so the real
            # matmul stream starts warm and never throttles.
            nc.gpsimd.memset(wscr[:], 0)
            nc.scalar.dma_start(w1_sb[:], w1_d[:])
            for i in range(NWARM):
                wps = pspool.tile([128, PSF], dt.float32, name="wps", tag="ps")
                nc.tensor.matmul(
                    wps[:, 0:RUN],
                    wscr[:, 0:NCH, 0:128],
                    wscr[:, 0:NCH, 0:RUN],
                    start=True, stop=True,
                    perf_mode=mybir.MatmulPerfMode.DoubleRow,
                )

            # remaining params on the gpsimd queue; x loads own sync
            nc.gpsimd.dma_start(w2_sb[:], w2_d[:])
            nc.gpsimd.dma_start(t1_sb[:], t1_d[:])
            nc.gpsimd.dma_start(t1s_sb[:], t1s_d[:])
            nc.gpsimd.dma_start(xqtn_sb[:], xqtn_d[:])
            nc.gpsimd.dma_start(s2_sb[:], s2_d[:])
            nc.gpsimd.dma_start(b2_sb[:], b2_d[:])

            # load x + quantize into qa1 interiors
            x_sb = [[None] * NCH for _ in range(NP_)]

            def xq_image(n):
                # plane fills via DMA: no engine time. image 0's loads are
                # spread across queues to shorten the prologue. images
                # 0..XQV-1: is_gt chain on DVE (qa1 in {0..3}, pad 0);
                # others: ACT Sign on ScalarE + adds on Pool (qa1 stores
                # S = 2*qa-3 in {-3..3}, pad -3; conv1 uses t1s).
                sign_coded = n >= XQV
                for j in range(NCH):
                    xt = xpool.tile([128, H, W], dt.float32,
                                    name=f"x_{n}_{j}", tag="x")
                    eng = nc.scalar if (n == 0 and j == 1) else nc.sync
                    eng.dma_start(xt[:], x_d[n, j * 128:(j + 1) * 128, :, :])
                    x_sb[n][j] = xt
                feng = nc.scalar if n == 0 else nc.gpsimd
                feng.dma_start(qa1[n][:], zm3_d[:] if sign_coded else z0_d[:])
                feng.dma_start(qa2[n][:], z0_d[:])
                for j in range(NCH):
                    xt = x_sb[n][j]
                    qa_in = plane(qa1[n], j)[:, 1:1 + H, 1:1 + W]
                    if not sign_coded:
                        m1 = mpool.tile([128, H, W], dt.bfloat16, name="m1",
                                        tag="xv1")
                        m2 = mpool.tile([128, H, W], dt.bfloat16, name="m2",
                                        tag="xv2")
                        nc.vector.tensor_scalar(
                            m1[:], xt[:], XQT[0], None, ALU.is_gt)
                        nc.vector.scalar_tensor_tensor(
                            m2[:], xt[:], XQT[1], m1[:], ALU.is_gt, ALU.add)
                        nc.vector.scalar_tensor_tensor(
                            qa_in, xt[:], XQT[2], m2[:], ALU.is_gt, ALU.add)
                    else:
                        s1 = mpool.tile([128, H, W], dt.bfloat16, name="s1",
                                        tag="xg1")
                        s2m = mpool.tile([128, H, W], dt.bfloat16, name="s2m",
                                         tag="xg2")
                        s3 = mpool.tile([128, H, W], dt.bfloat16, name="s3",
                                        tag="xg3")
                        st = mpool.tile([128, H, W], dt.bfloat16, name="st",
                                        tag="xg4")
                        for k, sk in enumerate((s1, s2m, s3)):
                            nc.scalar.activation(
                                sk[:], xt[:],
                                mybir.ActivationFunctionType.Sign,
                                bias=xqtn_sb[:, k:k + 1])
                        nc.gpsimd.tensor_tensor(st[:], s1[:], s2m[:], ALU.add)
                        nc.gpsimd.tensor_tensor(qa_in, st[:], s3[:], ALU.add)

            def conv_mms(ps, w_sb, qa_n, h, co):
                for k in range(9):
                    dy, dx = divmod(k, 3)
                    off = (h * HALF + dy) * PW + dx
                    nc.tensor.matmul(
                        ps[:, 0:RUN],
                        w_sb[:, 0:NCH, k, co * 128:(co + 1) * 128],
                        qa_n[:, 0:NCH, off:off + RUN],
                        start=(k == 0), stop=(k == 8),
                        perf_mode=mybir.MatmulPerfMode.DoubleRow,
                    )

            def psum_tile(name):
                ps = pspool.tile([128, PSF], dt.float32, name=name, tag="ps")
                psv = ps[:].rearrange("p (r c) -> p r c", c=PW)[:, :, 0:W]
                return ps, psv

            # conv1 -> bn1 -> relu -> quant, folded into 3 per-channel
            # is_gt thresholds accumulated with fused STT ops on DVE
            def conv1_tile(n, h, co):
                ps, psv = psum_tile("ps1")
                conv_mms(ps, w1_sb, qa1[n], h, co)
                thr = t1s_sb if n >= XQV else t1_sb
                e1 = mpool.tile([128, HALF, W], dt.bfloat16, name="e1",
                                tag="e1")
                e2 = mpool.tile([128, HALF, W], dt.bfloat16, name="e2",
                                tag="e2")
                nc.vector.tensor_scalar(
                    e1[:], psv, thr[:, co, 0:1], None, ALU.is_gt)
                nc.vector.scalar_tensor_tensor(
                    e2[:], psv, thr[:, co, 1:2], e1[:], ALU.is_gt, ALU.add)
                nc.vector.scalar_tensor_tensor(
                    plane(qa2[n], co)[:, 1 + h * HALF:1 + h * HALF + HALF,
                                      1:1 + W],
                    psv, thr[:, co, 2:3], e2[:], ALU.is_gt, ALU.add)

            def conv1_image(n):
                for h in range(2):
                    for co in range(NCH):
                        conv1_tile(n, h, co)

            # conv2 -> bn2 -> +residual -> relu -> out:
            #   v = ps*s2 + x (DVE STT), out = relu(v + bb2) (ACT bias)
            def conv2_image(n):
                for h in range(2):
                    for co in range(NCH):
                        ps, psv = psum_tile("ps2")
                        conv_mms(ps, w2_sb, qa2[n], h, co)
                        v = upool.tile([128, HALF, W], dt.float32, name="v",
                                       tag="v")
                        o = opool.tile([128, HALF, W], dt.float32, name="o",
                                       tag="o")
                        nc.vector.scalar_tensor_tensor(
                            v[:], psv, s2_sb[:, co:co + 1],
                            x_sb[n][co][:, h * HALF:(h + 1) * HALF, :],
                            ALU.mult, ALU.add)
                        nc.scalar.activation(
                            o[:], v[:], mybir.ActivationFunctionType.Relu,
                            bias=b2_sb[:, co:co + 1])
                        nc.sync.dma_start(
                            y_d[n, co * 128:(co + 1) * 128,
                                h * HALF:(h + 1) * HALF, :],
                            o[:])

            # software-pipelined emission: decouple engine queues by image
            for n in range(NP_):
                xq_image(n)
                conv1_image(n)
                if n >= 1:
                    conv2_image(n - 1)
            conv2_image(NP_ - 1)

    nc.compile()
    return nc


_CACHED = None


def _get_program():
    global _CACHED
    if _CACHED is None:
        _CACHED = _build_program()
    return _CACHED


def kernel(x, w1, g1, b1, m1, v1, w2, g2, b2, m2, v2):
    _install_ntff_hook_shim()
    from concourse.bass_utils import run_bass_kernel_spmd

    x = np.asarray(x, np.float32)
    host = _host_arrays(w1, g1, b1, m1, v1, w2, g2, b2, m2, v2)

    xs = x.reshape(NCORES, NPER, C, H, W)
    in_maps = [{"x": np.ascontiguousarray(xs[c]), **host}
               for c in range(NCORES)]

    nc = _get_program()
    res = run_bass_kernel_spmd(
        nc, in_maps, core_ids=list(range(NCORES)),
        trace=bool(int(os.environ.get("KERNEL_TRACE", "0"))),
    )
    kernel.last_results = res
    y = np.concatenate([res.results[c]["y"][None] for c in range(NCORES)], 0)
    return np.ascontiguousarray(y.reshape(64, C, H, W).astype(np.float32))


# revision 19
# speedup vs baseline: 1.2422x; 1.2422x over previous
"""Trainium2 Bass kernel for a 2-bit-quantized (DoReFa) ResNet BasicBlock.

Full (unsharded) numpy inputs -> full numpy output.

Design:
  - batch (64) is sharded 8 images/core across 8 NeuronCores (pure data
    parallel, weights/BN params replicated).
  - 2-bit quantization makes every conv input an exact small integer:
    acts*3 in {0..3}, weights*3 in {-3,-1,1,3}. Those are exact in fp8e4
    and the PE accumulates in fp32, so both convs are bit-exact integer
    matmuls at fp8 DoubleRow speed. All scale factors (the /9, BN
    scale/shift) fold into per-channel epilogue constants on host.
  - each 3x3 conv = 9 shifted matmuls accumulated in PSUM over padded
    30-wide activation planes. fp8 DoubleRow contracts all 256 input
    channels per matmul. The moving operand is a flat 418-element run
    across 14 plane rows; 2 garbage columns per row land in ignored psum
    columns.
  - activations everywhere use the is_gt coding qa = sum_k 1[v > t_k] in
    {0..3} with zero padding (zero == quantized zero), so conv1 and conv2
    use the same per-channel affine folds and no border corrections.
  - epilogues are fused via tensor_scalar / scalar_tensor_tensor with
    per-partition AP scalars:
      conv1: qa2 = ((ps>t1a) + (ps>t1b)) + (ps>t1c)   -> 3 DVE ops
      conv2: v = ps*s2 + x (STT);  out = relu(v + bb2) (ACT bias)
      x-quant: same 3-op is_gt chain (gpsimd for most images).
  - a scratch-fed run of dummy DoubleRow matmuls at kernel start warms the
    PE HAM clock gate so the real matmul stream runs at full rate and
    overlaps the x load + quantization prologue.
  - weight quantization (tanh / global max / round) + BN folding is done
    on host: it is O(weights) = 0.6 MB, vs 118 GFLOP of conv on device.
"""

import os
import sys
import numpy as np


def _install_ntff_hook_shim():
    """Provide antenv.axon_hooks if the image lacks it, so
    run_bass_kernel_spmd(trace=True) can capture NTFF profiles through
    libaxon_pjrt.so. No-op if the real module exists or the .so is absent."""
    try:
        import antenv.axon_hooks  # noqa: F401
        return
    except ImportError:
        pass
    import contextlib
    import ctypes
    import types

    so_path = "/opt/axon/libaxon_pjrt.so"
    _hook = None
    if os.path.exists(so_path):
        try:
            lib = ctypes.CDLL(so_path)
        except OSError:
            lib = None
        if lib is not None and hasattr(lib, "axon_start_nrt_profile"):
            lib.axon_start_nrt_profile.argtypes = [
                ctypes.POINTER(ctypes.c_int64), ctypes.c_size_t]
            lib.axon_start_nrt_profile.restype = ctypes.c_int64
            lib.axon_stop_nrt_profile.argtypes = [ctypes.c_char_p]
            lib.axon_stop_nrt_profile.restype = ctypes.c_int64

            @contextlib.contextmanager
            def _hook(output_dir, device_ids):
                import jax
                jax.devices()
                if device_ids:
                    ids = (ctypes.c_int64 * len(device_ids))(*device_ids)
                    rc = lib.axon_start_nrt_profile(ids, len(device_ids))
                else:
                    rc = lib.axon_start_nrt_profile(None, 0)
                if rc != 0:
                    raise RuntimeError(f"axon_start_nrt_profile rc={rc}")
                try:
                    yield
                finally:
                    n = lib.axon_stop_nrt_profile(str(output_dir).encode())
                    print(f"profile: {n} file(s) written to {output_dir}",
                          file=sys.stderr)

    mod = types.ModuleType("antenv.axon_hooks")
    mod.get_axon_ntff_profile_hook = lambda: _hook
    mod.set_axon_ntff_profile_hook = lambda h: None
    sys.modules["antenv.axon_hooks"] = mod


NCORES = 8
NPER = 8          # images per core
C = 256
NCH = 2           # channel chunks of 128
H = W = 28
PH = H + 2        # padded plane 30x30
PW = 30           # plane row stride
QSTR = 960        # allocated plane stride (16B-aligned, >= PH*PW)
HALF = 14         # rows per psum tile
RUN = (HALF - 1) * PW + W   # 418-element flat moving-run per matmul
PSF = HALF * PW   # 420 psum columns (cols 28..29 of each row are garbage)
BN_EPS = 1e-5
NWARM = int(os.environ.get("KERNEL_NWARM", "12"))   # PE warm-up matmuls
XQV = int(os.environ.get("KERNEL_XQV", "2"))        # imgs with x-quant on DVE


def _quant_weight3(w):
    """Replicate reference _quant_weight in f32, scaled by 3 -> {-3,-1,1,3}."""
    w = np.asarray(w, np.float32)
    t = np.tanh(w)
    m = np.max(np.abs(t))
    t2 = t / (np.float32(2.0) * m) + np.float32(0.5)
    k = np.round(t2 * np.float32(3.0))          # round-half-even == jnp.round
    return (2.0 * k - 3.0).astype(np.float32)


def _fold_bn(g, b, m, v):
    inv = np.asarray(g, np.float64) / np.sqrt(np.asarray(v, np.float64) + BN_EPS)
    beta = np.asarray(b, np.float64) - np.asarray(m, np.float64) * inv
    return inv, beta


def _w_tiles(qw3, dt):
    # [O, I, 3, 3] -> [p=128, ci=2, k=9, O=256] so lhsT slices are
    # [128, 2, 128] interleaved chunks (fp8 DoubleRow).
    return np.ascontiguousarray(
        np.transpose(qw3.reshape(C, NCH, 128, 9), (2, 1, 3, 0))
    ).astype(dt)


def _host_arrays(w1, g1, b1, m1, v1, w2, g2, b2, m2, v2):
    from concourse import mybir
    qw3_1 = _quant_weight3(w1)
    qw3_2 = _quant_weight3(w2)
    inv1, beta1 = _fold_bn(g1, b1, m1, v1)
    inv2, beta2 = _fold_bn(g2, b2, m2, v2)

    act_np = mybir.dt.np(mybir.dt.float8e4)
    w1t = _w_tiles(qw3_1, act_np)
    w2t = _w_tiles(qw3_2, act_np)

    # conv1 psum P1 = 9*conv_true (exact int); y = P1*inv1/9 + beta1.
    # quant level k iff y > tau_k = (2k-1)/6, i.e. P1 > (tau_k-beta1)*9/inv1
    # (inv1 > 0 given g1=1, v1 > 0), so qa2 = sum_k is_gt(P1, t1_k).
    assert np.all(inv1 > 0), "bn1 scale must be positive for threshold fold"
    taus = np.array([1.0, 3.0, 5.0]) / 6.0
    t1 = ((taus[None, :] - beta1[:, None]) * 9.0 / inv1[:, None])  # [C, 3]
    # sign-coded images store S1 = 2*qa1-3 (pad -3), so P1s = 2*P1 - 3*K1f
    # and the thresholds become 2*t1 - 3*K1f per output channel.
    k1f = qw3_1.reshape(C, -1).sum(axis=1)[:, None]                # [C, 1]
    t1s = 2.0 * t1 - 3.0 * k1f

    def fold_t(t):
        return np.ascontiguousarray(
            t.reshape(NCH, 128, 3).transpose(1, 0, 2)).astype(np.float32)

    # conv2 on qa2 in {0..3} with zero padding: P2 = 9*conv2_true, so
    # y2 = P2*(inv2/9) + beta2 and out = relu(y2 + x).
    s2 = np.ascontiguousarray(
        (inv2 / 9.0).reshape(NCH, 128).T).astype(np.float32)
    bb2 = np.ascontiguousarray(
        beta2.reshape(NCH, 128).T).astype(np.float32)
    z0 = np.zeros((128, NCH, QSTR), act_np)
    zm3 = np.full((128, NCH, QSTR), -3.0, act_np)
    xqtn = np.broadcast_to(
        -np.array([1.0, 3.0, 5.0], np.float32) / 6.0, (128, 3)).copy()
    return {"w1t": w1t, "w2t": w2t, "t1": fold_t(t1), "t1s": fold_t(t1s),
            "s2": s2, "bb2": bb2, "z0": z0, "zm3": zm3, "xqtn": xqtn}


def _build_program(nper=NPER):
    from concourse import bacc, tile, mybir
    dt = mybir.dt
    dt_act = dt.float8e4
    ALU = mybir.AluOpType

    nc = bacc.Bacc("TRN2", target_bir_lowering=False, debug=False,
                   num_devices=NCORES)
    NP_ = nper

    x_d = nc.dram_tensor("x", [NP_, C, H, W], dt.float32, kind="ExternalInput")
    w1_d = nc.dram_tensor("w1t", [128, NCH, 9, C], dt_act, kind="ExternalInput")
    w2_d = nc.dram_tensor("w2t", [128, NCH, 9, C], dt_act, kind="ExternalInput")
    t1_d = nc.dram_tensor("t1", [128, NCH, 3], dt.float32, kind="ExternalInput")
    t1s_d = nc.dram_tensor("t1s", [128, NCH, 3], dt.float32,
                           kind="ExternalInput")
    s2_d = nc.dram_tensor("s2", [128, NCH], dt.float32, kind="ExternalInput")
    b2_d = nc.dram_tensor("bb2", [128, NCH], dt.float32, kind="ExternalInput")
    z0_d = nc.dram_tensor("z0", [128, NCH, QSTR], dt_act, kind="ExternalInput")
    zm3_d = nc.dram_tensor("zm3", [128, NCH, QSTR], dt_act,
                           kind="ExternalInput")
    xqtn_d = nc.dram_tensor("xqtn", [128, 3], dt.float32,
                            kind="ExternalInput")
    y_d = nc.dram_tensor("y", [NP_, C, H, W], dt.float32, kind="ExternalOutput")

    XQT = [1.0 / 6.0, 3.0 / 6.0, 5.0 / 6.0]   # act-quant thresholds for x

    with tile.TileContext(nc) as tc:
        with (
            tc.tile_pool(name="wpool", bufs=1) as wpool,
            tc.tile_pool(name="xpool", bufs=2 * NP_) as xpool,
            tc.tile_pool(name="qpool", bufs=NP_) as qpool,
            tc.tile_pool(name="mpool", bufs=4) as mpool,
            tc.tile_pool(name="upool", bufs=4) as upool,
            tc.tile_pool(name="opool", bufs=4) as opool,
            tc.tile_pool(name="pspool", bufs=8, space="PSUM") as pspool,
        ):
            w1_sb = wpool.tile([128, NCH, 9, C], dt_act, name="w1sb")
            w2_sb = wpool.tile([128, NCH, 9, C], dt_act, name="w2sb")
            t1_sb = wpool.tile([128, NCH, 3], dt.float32, name="t1sb")
            t1s_sb = wpool.tile([128, NCH, 3], dt.float32, name="t1ssb")
            xqtn_sb = wpool.tile([128, 3], dt.float32, name="xqtnsb")
            s2_sb = wpool.tile([128, NCH], dt.float32, name="s2sb")
            b2_sb = wpool.tile([128, NCH], dt.float32, name="b2sb")
            wscr = wpool.tile([128, NCH, 512], dt_act, name="wscr")
            # zero-padded quantized-activation planes (flat, per image);
            # both hold is_gt sums in {0..3}, zero is the quantized zero.
            qa1 = [qpool.tile([128, NCH, QSTR], dt_act, name=f"qa1_{n}",
                              tag="qa1") for n in range(NP_)]
            qa2 = [qpool.tile([128, NCH, QSTR], dt_act, name=f"qa2_{n}",
                              tag="qa2") for n in range(NP_)]

            def plane(qa_t, j):
                return qa_t[:, j, :].rearrange("p (r c) -> p r c", c=PW)

            # --- PE warm-up: scratch-fed DoubleRow matmuls keep the HAM
            # clock gate open while the x prologue runs, so the real
            # matmul stream starts warm and never throttles.
            nc.gpsimd.memset(wscr[:], 0)
            nc.scalar.dma_start(w1_sb[:], w1_d[:])
            for i in range(NWARM):
                wps = pspool.tile([128, PSF], dt.float32, name="wps", tag="ps")
                nc.tensor.matmul(
                    wps[:, 0:RUN],
                    wscr[:, 0:NCH, 0:128],
                    wscr[:, 0:NCH, 0:RUN],
                    start=True, stop=True,
                    perf_mode=mybir.MatmulPerfMode.DoubleRow,
                )

            # remaining params on the gpsimd queue; x loads own sync
            nc.gpsimd.dma_start(w2_sb[:], w2_d[:])
            nc.gpsimd.dma_start(t1_sb[:], t1_d[:])
            nc.gpsimd.dma_start(t1s_sb[:], t1s_d[:])
            nc.gpsimd.dma_start(xqtn_sb[:], xqtn_d[:])
            nc.gpsimd.dma_start(s2_sb[:], s2_d[:])
            nc.gpsimd.dma_start(b2_sb[:], b2_d[:])

            # load x + quantize into qa1 interiors
            x_sb = [[None] * NCH for _ in range(NP_)]

            def xq_image(n):
                # plane fills via DMA: no engine time. image 0's loads are
                # spread across queues to shorten the prologue. images
                # 0..XQV-1: is_gt chain on DVE (qa1 in {0..3}, pad 0);
                # others: ACT Sign on ScalarE + adds on Pool (qa1 stores
                # S = 2*qa-3 in {-3..3}, pad -3; conv1 uses t1s).
                sign_coded = n >= XQV
                for j in range(NCH):
                    xt = xpool.tile([128, H, W], dt.float32,
                                    name=f"x_{n}_{j}", tag="x")
                    nc.sync.dma_start(xt[:],
                                      x_d[n, j * 128:(j + 1) * 128, :, :])
                    x_sb[n][j] = xt
                feng = nc.gpsimd
                feng.dma_start(qa1[n][:], zm3_d[:] if sign_coded else z0_d[:])
                feng.dma_start(qa2[n][:], z0_d[:])
                for j in range(NCH):
                    xt = x_sb[n][j]
                    qa_in = plane(qa1[n], j)[:, 1:1 + H, 1:1 + W]
                    if not sign_coded:
                        m1 = mpool.tile([128, H, W], dt.bfloat16, name="m1",
                                        tag="xv1")
                        m2 = mpool.tile([128, H, W], dt.bfloat16, name="m2",
                                        tag="xv2")
                        nc.vector.tensor_scalar(
                            m1[:], xt[:], XQT[0], None, ALU.is_gt)
                        nc.vector.scalar_tensor_tensor(
                            m2[:], xt[:], XQT[1], m1[:], ALU.is_gt, ALU.add)
                        nc.vector.scalar_tensor_tensor(
                            qa_in, xt[:], XQT[2], m2[:], ALU.is_gt, ALU.add)
                    else:
                        s1 = mpool.tile([128, H, W], dt.bfloat16, name="s1",
                                        tag="xg1")
                        s2m = mpool.tile([128, H, W], dt.bfloat16, name="s2m",
                                         tag="xg2")
                        s3 = mpool.tile([128, H, W], dt.bfloat16, name="s3",
                                        tag="xg3")
                        st = mpool.tile([128, H, W], dt.bfloat16, name="st",
                                        tag="xg4")
                        for k, sk in enumerate((s1, s2m, s3)):
                            nc.scalar.activation(
                                sk[:], xt[:],
                                mybir.ActivationFunctionType.Sign,
                                bias=xqtn_sb[:, k:k + 1])
                        nc.gpsimd.tensor_tensor(st[:], s1[:], s2m[:], ALU.add)
                        nc.gpsimd.tensor_tensor(qa_in, st[:], s3[:], ALU.add)

            def conv_mms2(pss, w_sb, qa_n, co):
                # tap-major over both h-halves: consecutive matmuls share
                # one stationary lhsT so the PE's background weight buffer
                # hides the (DoubleRow, 162ns) LDWEIGHTS behind the 2x177ns
                # of moving-stream time instead of serializing with it.
                for k in range(9):
                    dy, dx = divmod(k, 3)
                    for h in range(2):
                        off = (h * HALF + dy) * PW + dx
                        nc.tensor.matmul(
                            pss[h][:, 0:RUN],
                            w_sb[:, 0:NCH, k, co * 128:(co + 1) * 128],
                            qa_n[:, 0:NCH, off:off + RUN],
                            start=(k == 0), stop=(k == 8),
                            perf_mode=mybir.MatmulPerfMode.DoubleRow,
                        )

            def psum_tile(name):
                ps = pspool.tile([128, PSF], dt.float32, name=name, tag="ps")
                psv = ps[:].rearrange("p (r c) -> p r c", c=PW)[:, :, 0:W]
                return ps, psv

            # conv1 -> bn1 -> relu -> quant, folded into 3 per-channel
            # is_gt thresholds accumulated with fused STT ops on DVE
            def conv1_image(n):
                thr = t1s_sb if n >= XQV else t1_sb
                for co in range(NCH):
                    pss, psvs = zip(psum_tile("ps1a"), psum_tile("ps1b"))
                    conv_mms2(pss, w1_sb, qa1[n], co)
                    for h in range(2):
                        psv = psvs[h]
                        e1 = mpool.tile([128, HALF, W], dt.bfloat16,
                                        name="e1", tag="e1")
                        e2 = mpool.tile([128, HALF, W], dt.bfloat16,
                                        name="e2", tag="e2")
                        nc.vector.tensor_scalar(
                            e1[:], psv, thr[:, co, 0:1], None, ALU.is_gt)
                        nc.vector.scalar_tensor_tensor(
                            e2[:], psv, thr[:, co, 1:2], e1[:],
                            ALU.is_gt, ALU.add)
                        nc.vector.scalar_tensor_tensor(
                            plane(qa2[n], co)[:, 1 + h * HALF:
                                              1 + h * HALF + HALF, 1:1 + W],
                            psv, thr[:, co, 2:3], e2[:], ALU.is_gt, ALU.add)

            # conv2 -> bn2 -> +residual -> relu -> out:
            #   v = ps*s2 + x (DVE STT), out = relu(v + bb2) (ACT bias)
            def conv2_image(n):
                for co in range(NCH):
                    pss, psvs = zip(psum_tile("ps2a"), psum_tile("ps2b"))
                    conv_mms2(pss, w2_sb, qa2[n], co)
                    for h in range(2):
                        v = upool.tile([128, HALF, W], dt.float32, name="v",
                                       tag="v")
                        o = opool.tile([128, HALF, W], dt.float32, name="o",
                                       tag="o")
                        nc.vector.scalar_tensor_tensor(
                            v[:], psvs[h], s2_sb[:, co:co + 1],
                            x_sb[n][co][:, h * HALF:(h + 1) * HALF, :],
                            ALU.mult, ALU.add)
                        nc.scalar.activation(
                            o[:], v[:], mybir.ActivationFunctionType.Relu,
                            bias=b2_sb[:, co:co + 1])
                        nc.sync.dma_start(
                            y_d[n, co * 128:(co + 1) * 128,
                                h * HALF:(h + 1) * HALF, :],
                            o[:])

            # software-pipelined emission: decouple engine queues by image
            for n in range(NP_):
                xq_image(n)
                conv1_image(n)
                if n >= 1:
                    conv2_image(n - 1)
            conv2_image(NP_ - 1)

    nc.compile()
    return nc


_CACHED = None


def _get_program():
    global _CACHED
    if _CACHED is None:
        _CACHED = _build_program()
    return _CACHED


def kernel(x, w1, g1, b1, m1, v1, w2, g2, b2, m2, v2):
    _install_ntff_hook_shim()
    from concourse.bass_utils import run_bass_kernel_spmd

    x = np.asarray(x, np.float32)
    host = _host_arrays(w1, g1, b1, m1, v1, w2, g2, b2, m2, v2)

    xs = x.reshape(NCORES, NPER, C, H, W)
    in_maps = [{"x": np.ascontiguousarray(xs[c]), **host}
               for c in range(NCORES)]

    nc = _get_program()
    res = run_bass_kernel_spmd(
        nc, in_maps, core_ids=list(range(NCORES)),
        trace=bool(int(os.environ.get("KERNEL_TRACE", "0"))),
    )
    kernel.last_results = res
    y = np.concatenate([res.results[c]["y"][None] for c in range(NCORES)], 0)
    return np.ascontiguousarray(y.reshape(64, C, H, W).astype(np.float32))
